# revision 5
# baseline (speedup 1.0000x reference)
"""Trainium2 Bass kernel for nn_CNNFromScratch (dense 1-D CNN + MLP head).

Strategy
--------
Pure data parallelism: the batch axis (8192) is split across 8 NeuronCores
(1024 samples each); conv kernels and MLP weights are replicated.

Per core, everything is expressed as TensorE matmuls with the contraction
(input channels x taps) on the partition axis:

  - x is pre-laid-out on host as per-(tile, chunk) contiguous bf16 blocks
    (128 channels, w-major * batch free), so each chunk DMA is 128
    descriptors of 20*bt*2-byte contiguous runs (full HBM bandwidth, fast
    first-chunk arrival, ~100 descriptors/queue total).
  - All DMAs are issued unchained in consumption order: the 16 DMA rings
    process descriptors FIFO per queue, so completions come back in issue
    order with no inter-transfer dead time (an explicit dep chain costs
    ~2.2us per link in semaphore propagation + DGE re-trigger).
  - conv_k == sum over taps of  W_tap^T @ x[:, :, w+tap]  accumulated in PSUM.
  - Activations stay on-chip (SBUF, bf16) between layers; layout is
    (C_out partitions, w-major * batch free), which feeds the next conv's
    matmuls with plain contiguous slices.
  - maxpool = DVE tensor_max of two strided slices; MLP = accumulated
    matmuls over (channel, pooled-position) chunks.

Batch tiling: TILES=[256, 384, 384]. The head of the kernel is HBM-bound
(conv1 consumes x at ~455 GB/s vs ~330 GB/s delivered), so tile 0 is small
and its conv1 runs chunk-outer (q-outer): each x chunk is consumed right as
its DMA lands and its SBUF slot freed for prefetch. Its 9 output-position
groups need 9 concurrently-open PSUM accumulation banks (start_tensor_calc
zeroes a whole 2KB bank region, so groups cannot share a bank) — u=0..7
stay open across the chunk sweep and u=8 runs as a short trailing sweep.
Tile-0 conv1 uses 4-tap M=128 stationary blocks (one weight load per
stream window; M=64 parity pairs would need two loads per window, which
don't fit at N=256). Tiles 1-2 run position-outer M=64 parity pairs at
N=384 once x is resident.

Matmul inputs are bf16 (1 cycle/row on PE), accumulation is fp32 in PSUM.
"""

import sys

sys.path.insert(0, "/opt/trn_rl_repo")

import numpy as np
import ml_dtypes

N_CORES = 8
B, E, W = 8192, 512, 20
BC = B // N_CORES  # samples per core
TILES = [256, 384, 384]
assert sum(TILES) == BC
BT_MAX = max(TILES)

BF16 = ml_dtypes.bfloat16

_compiled = {}


def _build():
    import concourse.bass as bass
    from concourse import bacc, mybir
    import concourse.tile as tile

    dt = mybir.dt
    AF = mybir.ActivationFunctionType

    nc = bacc.Bacc(
        "TRN2",
        target_bir_lowering=False,
        debug=False,
        enable_asserts=False,
        num_devices=N_CORES,
    )

    # x: one contiguous (128, 20*bt) block per (tile, chunk), so the DMA is
    # 128 descriptors of 20*bt*2 contiguous bytes each.
    x_d = [
        nc.dram_tensor(f"x{t}", (4, 128, 20 * bt), dt.bfloat16, kind="ExternalInput").ap()
        for t, bt in enumerate(TILES)
    ]
    # 4-tap conv1 blocks for tile 0 (see _prep_inputs)
    w1t_d = nc.dram_tensor("w1t", (512, 4 * 128), dt.bfloat16, kind="ExternalInput").ap()
    w1_d = nc.dram_tensor("w1", (512, 3 * 64), dt.bfloat16, kind="ExternalInput").ap()
    w2_d = nc.dram_tensor("w2", (128, 6 * 128), dt.bfloat16, kind="ExternalInput").ap()
    w3_d = nc.dram_tensor("w3", (128, 7 * 256), dt.bfloat16, kind="ExternalInput").ap()
    m1_d = nc.dram_tensor("m1", (1024, 256), dt.bfloat16, kind="ExternalInput").ap()
    m2_d = nc.dram_tensor("m2", (256, 128), dt.bfloat16, kind="ExternalInput").ap()
    m3_d = nc.dram_tensor("m3", (128, 1), dt.bfloat16, kind="ExternalInput").ap()
    y_d = nc.dram_tensor("y", (1, BC), dt.float32, kind="ExternalOutput").ap()

    with tile.TileContext(nc) as tc:
        with (
            tc.tile_pool(name="sb", bufs=1) as sb,
            tc.tile_pool(name="ps", bufs=8, space="PSUM") as ps,
        ):
            # Every PSUM tile is a full 2KB bank: start_tensor_calc zeroes
            # bank-granular regions, so two groups must never share a bank.
            def ps_bank(name):
                return ps.tile([128, 512], dt.float32, tag="ps", name=name)

            # ---- conv1 weights first: PE can start as soon as x chunk 0 lands
            w1t_sb = []
            for q in range(4):
                t = sb.tile([128, 4 * 128], dt.bfloat16, tag=f"w1t_{q}")
                nc.scalar.dma_start(t[:], w1t_d[q * 128 : (q + 1) * 128, :])
                w1t_sb.append(t)
            w1_sb = []
            for q in range(4):
                t = sb.tile([128, 3 * 64], dt.bfloat16, tag=f"w1_{q}")
                nc.scalar.dma_start(t[:], w1_d[q * 128 : (q + 1) * 128, :])
                w1_sb.append(t)

            x_insts = {}

            def load_x_chunk(t, q, bt):
                xt = sb.tile(
                    [128, 20 * BT_MAX], dt.bfloat16, tag="x", bufs=6, name=f"x_{t}_{q}"
                )
                x_insts[(t, q)] = nc.sync.dma_start(xt[:, : 20 * bt], x_d[t][q]).ins
                return xt[:, : 20 * bt]

            # Warm the PE clock gate during the initial x DMA wait (dummy
            # matmuls on the already-loaded weights; results never read) and
            # pull the ACT Relu table load off the critical path.
            warm_in = sb.tile([128, 192], dt.bfloat16, tag="warm_in")
            nc.gpsimd.memset(warm_in[:], 0.0)
            warm_ps = ps_bank("warm_ps")
            for _ in range(40):
                nc.tensor.matmul(
                    warm_ps[0:64, 0:192],
                    warm_in[:, 0:64],
                    warm_in[:, :],
                    start=True,
                    stop=True,
                )
            warm_act = sb.tile([1, 1], dt.float32, tag="warm_act")
            nc.scalar.activation(warm_act[:], warm_in[0:1, 0:1], AF.Relu)

            # x tile 0 first (head of the ring FIFO). The bulk weights are
            # gated on x-t0-q0's completion (one dep hop each, no W->W
            # chaining) so their descriptors enqueue behind all of x-t0's in
            # the per-queue FIFOs: W lands right after x-t0, just before
            # conv2-t0 needs it, without delaying conv1's stream.
            x0 = [load_x_chunk(0, q, TILES[0]) for q in range(4)]

            def wload(bass_inst):
                tile.add_dep_helper(
                    bass_inst.ins, x_insts[(0, 0)], reason="bulk W after x tile 0"
                )

            w2_sb = sb.tile([128, 6 * 128], dt.bfloat16, tag="w2")
            wload(nc.scalar.dma_start(w2_sb[:], w2_d[:, :]))
            w3_sb = sb.tile([128, 7 * 256], dt.bfloat16, tag="w3")
            wload(nc.scalar.dma_start(w3_sb[:], w3_d[:, :]))
            m1_sb = []
            for wp in range(4):
                row = []
                for q in range(2):
                    t = sb.tile([128, 256], dt.bfloat16, tag=f"m1_{wp}_{q}")
                    r0 = wp * 256 + q * 128
                    wload(nc.scalar.dma_start(t[:], m1_d[r0 : r0 + 128, :]))
                    row.append(t)
                m1_sb.append(row)
            m2_sb = []
            for q in range(2):
                t = sb.tile([128, 128], dt.bfloat16, tag=f"m2_{q}")
                wload(nc.scalar.dma_start(t[:], m2_d[q * 128 : (q + 1) * 128, :]))
                m2_sb.append(t)
            m3_sb = sb.tile([128, 1], dt.bfloat16, tag="m3")
            wload(nc.scalar.dma_start(m3_sb[:], m3_d[:, :]))

            # ---- per-batch-tile pipeline ----
            boff = 0
            for ti, bt in enumerate(TILES):
                x_sb = x0 if ti == 0 else [load_x_chunk(ti, q, bt) for q in range(4)]

                # conv1: (B,512,20) -> relu -> (B,64,18)
                # h1 layout: position pairs u; even w'=2u on partitions 0-63,
                # odd w'=2u+1 on 64-127.
                h1 = sb.tile([128, 9 * bt], dt.bfloat16, tag="h1", bufs=2)

                if ti == 0:
                    # Chunk-outer with 4-tap M=128 stationary blocks: group u
                    # accumulates  sum_t w1t[t]^T @ x[2u+t]  (t=0..3), taps
                    # 0-2 feeding even-position channels on partitions 0-63
                    # and taps 1-3 odd ones on 64-127 (zero-padded edges).
                    # One weight load per matmul window -> no LDWEIGHTS
                    # exposure at N=256; one accumulation group per bank.
                    def conv1_q_outer(us, p1s):
                        for q in range(4):
                            for k in range(4):
                                for i, u in enumerate(us):
                                    nc.tensor.matmul(
                                        p1s[i][:, :bt],
                                        w1t_sb[q][:, k * 128 : (k + 1) * 128],
                                        x_sb[q][:, (2 * u + k) * bt : (2 * u + k + 1) * bt],
                                        start=(q == 0 and k == 0),
                                        stop=(q == 3 and k == 3),
                                    )
                        for i, u in enumerate(us):
                            nc.scalar.activation(
                                h1[:, u * bt : (u + 1) * bt], p1s[i][:, :bt], AF.Relu
                            )

                    conv1_q_outer(
                        list(range(8)), [ps_bank(f"p1_{u}") for u in range(8)]
                    )
                    conv1_q_outer([8], [ps_bank("p1_8")])
                else:
                    # Position-outer M=64 parity pairs (two PE column groups
                    # run concurrently); x is fully resident by this tile.
                    # The group checker doesn't model two col-groups in one
                    # bank, hence skip_group_check.
                    for u in range(9):
                        p1 = ps_bank("p1")
                        for q in range(4):
                            for k in range(3):
                                for par in range(2):
                                    nc.tensor.matmul(
                                        p1[par * 64 : (par + 1) * 64, :bt],
                                        w1_sb[q][:, k * 64 : (k + 1) * 64],
                                        x_sb[q][
                                            :,
                                            (2 * u + par + k) * bt : (2 * u + par + k + 1) * bt,
                                        ],
                                        start=(q == 0 and k == 0),
                                        stop=(q == 3 and k == 2),
                                        skip_group_check=True,
                                    )
                        nc.scalar.activation(
                            h1[:, u * bt : (u + 1) * bt], p1[:, :bt], AF.Relu
                        )

                # conv2: -> relu -> (B,128,14)
                # h1's parity-split layout lets adjacent taps fuse into one
                # full 128-row contraction (tap k on rows 0-63, tap k+1 on
                # 64-127), with zero-padded weight blocks at the edges so
                # every matmul is full-height: 3 matmuls per position
                # instead of 5. Host-prepped blocks (see _prep_inputs):
                #   even w': [k0;k1] [k2;k3] [k4;0 ]  at h1 cols t', t'+1, t'+2
                #   odd  w': [0;k0] [k1;k2] [k3;k4]   at h1 cols t', t'+1, t'+2
                h2 = sb.tile([128, 14 * bt], dt.bfloat16, tag="h2")
                for w in range(14):
                    t0 = w // 2
                    blk0 = 0 if w % 2 == 0 else 3
                    p2 = ps_bank("p2")
                    for j in range(3):
                        blk = blk0 + j
                        nc.tensor.matmul(
                            p2[:, :bt],
                            w2_sb[:, blk * 128 : (blk + 1) * 128],
                            h1[:, (t0 + j) * bt : (t0 + j + 1) * bt],
                            start=(j == 0),
                            stop=(j == 2),
                        )
                    nc.vector.tensor_relu(h2[:, w * bt : (w + 1) * bt], p2[:, :bt])

                # conv3: -> relu -> (B,256,8) as two 128-channel tiles
                h3 = [
                    sb.tile([128, 8 * bt], dt.bfloat16, tag=f"h3_{m}", name=f"h3_{m}")
                    for m in range(2)
                ]
                for w in range(8):
                    for m in range(2):
                        p3 = ps_bank("p3")
                        for k in range(7):
                            nc.tensor.matmul(
                                p3[:, :bt],
                                w3_sb[:, k * 256 + m * 128 : k * 256 + (m + 1) * 128],
                                h2[:, (w + k) * bt : (w + k + 1) * bt],
                                start=(k == 0),
                                stop=(k == 6),
                            )
                        nc.vector.tensor_relu(h3[m][:, w * bt : (w + 1) * bt], p3[:, :bt])

                # maxpool k=2 s=2: (B,256,8) -> (B,256,4)
                pooled = [
                    sb.tile([128, 4 * bt], dt.bfloat16, tag=f"pool_{m}", name=f"pool_{m}")
                    for m in range(2)
                ]
                for m in range(2):
                    for p in range(4):
                        nc.vector.tensor_max(
                            pooled[m][:, p * bt : (p + 1) * bt],
                            h3[m][:, (2 * p) * bt : (2 * p + 1) * bt],
                            h3[m][:, (2 * p + 1) * bt : (2 * p + 2) * bt],
                        )

                # mlp1: (B,1024)->(B,256), f = c*4 + wp
                g1 = [
                    sb.tile([128, bt], dt.bfloat16, tag=f"g1_{j}", name=f"g1_{j}")
                    for j in range(2)
                ]
                for j in range(2):
                    pm = ps_bank("pm1")
                    for wp in range(4):
                        for q in range(2):
                            nc.tensor.matmul(
                                pm[:, :bt],
                                m1_sb[wp][q][:, j * 128 : (j + 1) * 128],
                                pooled[q][:, wp * bt : (wp + 1) * bt],
                                start=(wp == 0 and q == 0),
                                stop=(wp == 3 and q == 1),
                            )
                    nc.vector.tensor_relu(g1[j][:], pm[:, :bt])

                # mlp2: (B,256)->(B,128)
                g2 = sb.tile([128, bt], dt.bfloat16, tag="g2")
                pm = ps_bank("pm2")
                for q in range(2):
                    nc.tensor.matmul(
                        pm[:, :bt], m2_sb[q][:], g1[q][:], start=(q == 0), stop=(q == 1)
                    )
                nc.vector.tensor_relu(g2[:], pm[:, :bt])

                # mlp3: (B,128)->(B,1)
                pm = ps_bank("pm3")
                nc.tensor.matmul(pm[0:1, :bt], m3_sb[:], g2[:], start=True, stop=True)
                y_sb = sb.tile([1, BT_MAX], dt.float32, tag="y_sb", bufs=2)
                nc.vector.tensor_copy(y_sb[:, :bt], pm[0:1, :bt])
                nc.sync.dma_start(y_d[:, boff : boff + bt], y_sb[:, :bt])

                boff += bt

    nc.compile()
    return nc


def _prep_inputs(x, kernel_1, kernel_2, kernel_3, mlp_weight_1, mlp_weight_2, mlp_weight_3):
    """Host-side sharding + layout prep. Returns in_maps (one dict per core)."""
    k1t = kernel_1.transpose(1, 2, 0).astype(np.float32)  # (512, 3, 64)
    w1 = np.ascontiguousarray(k1t.reshape(512, 3 * 64)).astype(BF16)
    # 4-tap M=128 conv1 blocks: block t columns 0-63 = W1[tap t] (even
    # output positions, taps 0-2), columns 64-127 = W1[tap t-1] (odd
    # positions, taps 1-3), zero at the edges.
    z64 = np.zeros((512, 64), np.float32)
    w1t = np.concatenate(
        [
            np.concatenate(
                [k1t[:, t] if t < 3 else z64, k1t[:, t - 1] if t >= 1 else z64],
                axis=1,
            )
            for t in range(4)
        ],
        axis=1,
    )
    w1t = np.ascontiguousarray(w1t).astype(BF16)  # (512, 4*128)
    # conv2 tap-pair blocks for the parity-split h1 layout: column block j is
    # a (128, 128) lhsT whose rows 0-63 multiply h1's even half and rows
    # 64-127 the odd half. Blocks 0-2 serve even output positions
    # ([k0;k1] [k2;k3] [k4;0]), blocks 3-5 odd ones ([0;k0] [k1;k2] [k3;k4]).
    k2t = kernel_2.transpose(1, 2, 0).astype(np.float32)  # (64, 5, 128)
    z = np.zeros((64, 128), np.float32)
    blocks = [
        np.concatenate([k2t[:, 0], k2t[:, 1]], axis=0),
        np.concatenate([k2t[:, 2], k2t[:, 3]], axis=0),
        np.concatenate([k2t[:, 4], z], axis=0),
        np.concatenate([z, k2t[:, 0]], axis=0),
        np.concatenate([k2t[:, 1], k2t[:, 2]], axis=0),
        np.concatenate([k2t[:, 3], k2t[:, 4]], axis=0),
    ]
    w2 = np.ascontiguousarray(np.concatenate(blocks, axis=1)).astype(BF16)
    w3 = np.ascontiguousarray(
        kernel_3.transpose(1, 2, 0).reshape(128, 7 * 256)
    ).astype(BF16)
    # W1 row f = c*4 + wp  ->  m1 row = wp*256 + c
    m1 = np.ascontiguousarray(
        mlp_weight_1.reshape(256, 4, 256).transpose(1, 0, 2).reshape(1024, 256)
    ).astype(BF16)
    m2 = mlp_weight_2.astype(BF16)
    m3 = mlp_weight_3.astype(BF16)

    xb = x.astype(BF16)
    in_maps = []
    for c in range(N_CORES):
        xc = xb[c * BC : (c + 1) * BC].transpose(1, 2, 0)  # (512, 20, BC)
        m = {
            "w1t": w1t, "w1": w1, "w2": w2, "w3": w3,
            "m1": m1, "m2": m2, "m3": m3,
        }
        boff = 0
        for t, bt in enumerate(TILES):
            m[f"x{t}"] = np.ascontiguousarray(
                xc[:, :, boff : boff + bt].reshape(4, 128, 20 * bt)
            )
            boff += bt
        in_maps.append(m)
    return in_maps


def run(inputs, trace=False, **kw):
    """Compile (cached), run on 8 cores, return (y_full, BassKernelResults)."""
    from concourse import bass_utils

    if "nc" not in _compiled:
        _compiled["nc"] = _build()
    nc = _compiled["nc"]
    in_maps = _prep_inputs(**inputs)
    res = bass_utils.run_bass_kernel_spmd(
        nc, in_maps, core_ids=list(range(N_CORES)), trace=trace, **kw
    )
    y = np.concatenate(
        [res.results[c]["y"].reshape(BC, 1) for c in range(N_CORES)], axis=0
    )
    return y.astype(np.float32), res


def kernel(**inputs):
    inputs = {k: np.asarray(v) for k, v in inputs.items()}
    y, _ = run(inputs)
    return y


if __name__ == "__main__":
    rng = np.random.default_rng(0)
    inputs = {
        "x": rng.standard_normal((B, E, W), dtype=np.float32),
        "kernel_1": rng.standard_normal((64, 512, 3), dtype=np.float32),
        "kernel_2": rng.standard_normal((128, 64, 5), dtype=np.float32),
        "kernel_3": rng.standard_normal((256, 128, 7), dtype=np.float32),
        "mlp_weight_1": rng.standard_normal((1024, 256), dtype=np.float32),
        "mlp_weight_2": rng.standard_normal((256, 128), dtype=np.float32),
        "mlp_weight_3": rng.standard_normal((128, 1), dtype=np.float32),
    }
    y = kernel(**inputs)
    print("out", y.shape, y.dtype, y[:4, 0])


# revision 6
# speedup vs baseline: 1.1809x; 1.1809x over previous
"""Trainium2 Bass kernel for nn_CNNFromScratch (dense 1-D CNN + MLP head).

Strategy
--------
Pure data parallelism: the batch axis (8192) is split across 8 NeuronCores
(1024 samples each); conv kernels and MLP weights are replicated.

Per core, everything is expressed as TensorE matmuls with the contraction
(input channels x taps) on the partition axis:

  - x is pre-laid-out on host as per-(tile, chunk) contiguous bf16 blocks
    (128 channels, w-major * batch free), so each chunk DMA is 128
    descriptors of 20*bt*2-byte contiguous runs (full HBM bandwidth, fast
    first-chunk arrival, ~100 descriptors/queue total).
  - All x DMAs are issued unchained in consumption order: the 16 DMA rings
    process descriptors FIFO per queue, so completions come back in issue
    order with no inter-transfer dead time (an explicit dep chain costs
    ~2.2us per link in semaphore propagation + DGE re-trigger). The bulk
    weights are gated on x-t0-q0 completion (one hop each, no W->W chain)
    so they enqueue behind x-t0 and land just before conv2-t0 needs them.
  - conv_k == sum over taps of  W_tap^T @ x[:, :, w+tap]  accumulated in PSUM.
  - Activations stay on-chip (SBUF, bf16) between layers; layout is
    (C_out partitions, w-major * batch free), which feeds the next conv's
    matmuls with plain contiguous slices.
  - maxpool = DVE tensor_max of two strided slices; MLP = accumulated
    matmuls over (channel, pooled-position) chunks.

All matmuls stream N=512 (bt=512): shorter streams cannot hide the
per-matmul LDWEIGHTS (~95ns), and exposed weight loads both waste PE time
and correlate with the PE settling at a reduced clock. Tile 0's conv1 runs
chunk-outer within position blocks so the PE starts as soon as the first
x chunk lands.

Matmul inputs are bf16 (1 cycle/row on PE), accumulation is fp32 in PSUM.
"""

import sys

sys.path.insert(0, "/opt/trn_rl_repo")

import numpy as np
import ml_dtypes

N_CORES = 8
B, E, W = 8192, 512, 20
BC = B // N_CORES  # samples per core
TILES = [512, 512]
assert sum(TILES) == BC
BT_MAX = max(TILES)

BF16 = ml_dtypes.bfloat16

_compiled = {}


def _build():
    import concourse.bass as bass
    from concourse import bacc, mybir
    import concourse.tile as tile

    dt = mybir.dt
    AF = mybir.ActivationFunctionType

    nc = bacc.Bacc(
        "TRN2",
        target_bir_lowering=False,
        debug=False,
        enable_asserts=False,
        num_devices=N_CORES,
    )

    # x: one contiguous (128, 20*bt) block per (tile, chunk), so the DMA is
    # 128 descriptors of 20*bt*2 contiguous bytes each.
    x_d = [
        nc.dram_tensor(f"x{t}", (4, 128, 20 * bt), dt.bfloat16, kind="ExternalInput").ap()
        for t, bt in enumerate(TILES)
    ]
    w1_d = nc.dram_tensor("w1", (512, 3 * 64), dt.bfloat16, kind="ExternalInput").ap()
    w2_d = nc.dram_tensor("w2", (128, 6 * 128), dt.bfloat16, kind="ExternalInput").ap()
    w3_d = nc.dram_tensor("w3", (128, 7 * 256), dt.bfloat16, kind="ExternalInput").ap()
    m1_d = nc.dram_tensor("m1", (1024, 256), dt.bfloat16, kind="ExternalInput").ap()
    m2_d = nc.dram_tensor("m2", (256, 128), dt.bfloat16, kind="ExternalInput").ap()
    m3_d = nc.dram_tensor("m3", (128, 1), dt.bfloat16, kind="ExternalInput").ap()
    y_d = nc.dram_tensor("y", (1, BC), dt.float32, kind="ExternalOutput").ap()

    with tile.TileContext(nc) as tc:
        with (
            tc.tile_pool(name="sb", bufs=1) as sb,
            tc.tile_pool(name="ps", bufs=8, space="PSUM") as ps,
        ):
            # ---- conv1 weights first: PE can start as soon as x chunk 0 lands
            w1_sb = []
            for q in range(4):
                t = sb.tile([128, 3 * 64], dt.bfloat16, tag=f"w1_{q}")
                nc.scalar.dma_start(t[:], w1_d[q * 128 : (q + 1) * 128, :])
                w1_sb.append(t)

            x_insts = {}

            def load_x_chunk(t, q, bt):
                xt = sb.tile(
                    [128, 20 * BT_MAX], dt.bfloat16, tag="x", bufs=5, name=f"x_{t}_{q}"
                )
                x_insts[(t, q)] = nc.sync.dma_start(xt[:, : 20 * bt], x_d[t][q]).ins
                return xt[:, : 20 * bt]

            # Warm the PE clock gate during the initial x DMA wait (dummy
            # matmuls on the already-loaded w1 tile; results never read) and
            # pull the ACT Relu table load off the critical path.
            warm_in = sb.tile([128, 192], dt.bfloat16, tag="warm_in")
            nc.gpsimd.memset(warm_in[:], 0.0)
            warm_ps = ps.tile([128, 512], dt.float32, tag="ps", name="warm_ps")
            for _ in range(55):
                nc.tensor.matmul(
                    warm_ps[0:64, 0:192],
                    warm_in[:, 0:64],
                    warm_in[:, :],
                    start=True,
                    stop=True,
                )
            warm_act = sb.tile([1, 1], dt.float32, tag="warm_act")
            nc.scalar.activation(warm_act[:], warm_in[0:1, 0:1], AF.Relu)

            # x tile 0 at the head of the ring FIFO; bulk weights gated on
            # x-t0-q0 so their descriptors enqueue behind all of x-t0's.
            x0 = [load_x_chunk(0, q, TILES[0]) for q in range(4)]

            def wload(bass_inst):
                tile.add_dep_helper(
                    bass_inst.ins, x_insts[(0, 0)], reason="bulk W after x tile 0"
                )

            w2_sb = sb.tile([128, 6 * 128], dt.bfloat16, tag="w2")
            wload(nc.scalar.dma_start(w2_sb[:], w2_d[:, :]))
            w3_sb = sb.tile([128, 7 * 256], dt.bfloat16, tag="w3")
            wload(nc.scalar.dma_start(w3_sb[:], w3_d[:, :]))
            m1_sb = []
            for wp in range(4):
                row = []
                for q in range(2):
                    t = sb.tile([128, 256], dt.bfloat16, tag=f"m1_{wp}_{q}")
                    r0 = wp * 256 + q * 128
                    wload(nc.scalar.dma_start(t[:], m1_d[r0 : r0 + 128, :]))
                    row.append(t)
                m1_sb.append(row)
            m2_sb = []
            for q in range(2):
                t = sb.tile([128, 128], dt.bfloat16, tag=f"m2_{q}")
                wload(nc.scalar.dma_start(t[:], m2_d[q * 128 : (q + 1) * 128, :]))
                m2_sb.append(t)
            m3_sb = sb.tile([128, 1], dt.bfloat16, tag="m3")
            wload(nc.scalar.dma_start(m3_sb[:], m3_d[:, :]))

            # ---- per-batch-tile pipeline ----
            boff = 0
            for ti, bt in enumerate(TILES):
                x_sb = x0 if ti == 0 else [load_x_chunk(ti, q, bt) for q in range(4)]

                # conv1: (B,512,20) -> relu -> (B,64,18)
                # Output positions are packed in pairs: even w on PSUM/SBUF
                # partitions 0-63, odd w on 64-127. The two M=64 accumulation
                # groups land on different PE column groups and execute
                # concurrently (~2x conv1 throughput).
                # The group checker doesn't model a two-col-group interleave
                # in one bank, hence skip_group_check.
                h1 = sb.tile([128, 9 * bt], dt.bfloat16, tag="h1")

                def conv1_mms(p1, u, q):
                    for k in range(3):
                        for par in range(2):
                            nc.tensor.matmul(
                                p1[par * 64 : (par + 1) * 64, :],
                                w1_sb[q][:, k * 64 : (k + 1) * 64],
                                x_sb[q][
                                    :,
                                    (2 * u + par + k) * bt : (2 * u + par + k + 1) * bt,
                                ],
                                start=(q == 0 and k == 0),
                                stop=(q == 3 and k == 2),
                                skip_group_check=True,
                            )

                if ti == 0:
                    # Chunk-outer: all matmuls for chunk q across a block of
                    # output pairs before moving to chunk q+1, so the PE
                    # starts when the first c-chunk DMA lands instead of
                    # waiting for all four.
                    for u0, u1 in ((0, 8), (8, 9)):
                        p1s = [
                            ps.tile([128, bt], dt.float32, tag="ps", name=f"p1_{u}")
                            for u in range(u0, u1)
                        ]
                        for q in range(4):
                            for u in range(u0, u1):
                                conv1_mms(p1s[u - u0], u, q)
                        for u in range(u0, u1):
                            nc.scalar.activation(
                                h1[:, u * bt : (u + 1) * bt], p1s[u - u0][:], AF.Relu
                            )
                else:
                    for u in range(9):
                        p1 = ps.tile([128, bt], dt.float32, tag="ps")
                        for q in range(4):
                            conv1_mms(p1, u, q)
                        nc.scalar.activation(
                            h1[:, u * bt : (u + 1) * bt], p1[:], AF.Relu
                        )

                # conv2: -> relu -> (B,128,14)
                # h1's parity-split layout lets adjacent taps fuse into one
                # full 128-row contraction (tap k on rows 0-63, tap k+1 on
                # 64-127), with zero-padded weight blocks at the edges so
                # every matmul is full-height: 3 matmuls per position
                # instead of 5. Host-prepped blocks (see _prep_inputs):
                #   even w': [k0;k1] [k2;k3] [k4;0 ]  at h1 cols t', t'+1, t'+2
                #   odd  w': [0;k0] [k1;k2] [k3;k4]   at h1 cols t', t'+1, t'+2
                h2 = sb.tile([128, 14 * bt], dt.bfloat16, tag="h2")
                for w in range(14):
                    t0 = w // 2
                    blk0 = 0 if w % 2 == 0 else 3
                    p2 = ps.tile([128, bt], dt.float32, tag="ps")
                    for j in range(3):
                        blk = blk0 + j
                        nc.tensor.matmul(
                            p2[:],
                            w2_sb[:, blk * 128 : (blk + 1) * 128],
                            h1[:, (t0 + j) * bt : (t0 + j + 1) * bt],
                            start=(j == 0),
                            stop=(j == 2),
                        )
                    nc.vector.tensor_relu(h2[:, w * bt : (w + 1) * bt], p2[:])

                # conv3: -> relu -> (B,256,8) as two 128-channel tiles
                h3 = [
                    sb.tile([128, 8 * bt], dt.bfloat16, tag=f"h3_{m}", name=f"h3_{m}")
                    for m in range(2)
                ]
                for w in range(8):
                    for m in range(2):
                        p3 = ps.tile([128, bt], dt.float32, tag="ps")
                        for k in range(7):
                            nc.tensor.matmul(
                                p3[:],
                                w3_sb[:, k * 256 + m * 128 : k * 256 + (m + 1) * 128],
                                h2[:, (w + k) * bt : (w + k + 1) * bt],
                                start=(k == 0),
                                stop=(k == 6),
                            )
                        nc.vector.tensor_relu(h3[m][:, w * bt : (w + 1) * bt], p3[:])

                # maxpool k=2 s=2: (B,256,8) -> (B,256,4)
                pooled = [
                    sb.tile([128, 4 * bt], dt.bfloat16, tag=f"pool_{m}", name=f"pool_{m}")
                    for m in range(2)
                ]
                for m in range(2):
                    for p in range(4):
                        nc.vector.tensor_max(
                            pooled[m][:, p * bt : (p + 1) * bt],
                            h3[m][:, (2 * p) * bt : (2 * p + 1) * bt],
                            h3[m][:, (2 * p + 1) * bt : (2 * p + 2) * bt],
                        )

                # mlp1: (B,1024)->(B,256), f = c*4 + wp
                g1 = [
                    sb.tile([128, bt], dt.bfloat16, tag=f"g1_{j}", name=f"g1_{j}")
                    for j in range(2)
                ]
                for j in range(2):
                    pm = ps.tile([128, bt], dt.float32, tag="ps")
                    for wp in range(4):
                        for q in range(2):
                            nc.tensor.matmul(
                                pm[:],
                                m1_sb[wp][q][:, j * 128 : (j + 1) * 128],
                                pooled[q][:, wp * bt : (wp + 1) * bt],
                                start=(wp == 0 and q == 0),
                                stop=(wp == 3 and q == 1),
                            )
                    nc.vector.tensor_relu(g1[j][:], pm[:])

                # mlp2: (B,256)->(B,128)
                g2 = sb.tile([128, bt], dt.bfloat16, tag="g2")
                pm = ps.tile([128, bt], dt.float32, tag="ps")
                for q in range(2):
                    nc.tensor.matmul(
                        pm[:], m2_sb[q][:], g1[q][:], start=(q == 0), stop=(q == 1)
                    )
                nc.vector.tensor_relu(g2[:], pm[:])

                # mlp3: (B,128)->(B,1)
                pm = ps.tile([1, bt], dt.float32, tag="ps")
                nc.tensor.matmul(pm[:], m3_sb[:], g2[:], start=True, stop=True)
                y_sb = sb.tile([1, BT_MAX], dt.float32, tag="y_sb", bufs=2)
                nc.vector.tensor_copy(y_sb[:, :bt], pm[:])
                nc.sync.dma_start(y_d[:, boff : boff + bt], y_sb[:, :bt])

                boff += bt

    nc.compile()
    return nc


def _prep_inputs(x, kernel_1, kernel_2, kernel_3, mlp_weight_1, mlp_weight_2, mlp_weight_3):
    """Host-side sharding + layout prep. Returns in_maps (one dict per core)."""
    w1 = np.ascontiguousarray(
        kernel_1.transpose(1, 2, 0).reshape(512, 3 * 64)
    ).astype(BF16)
    # conv2 tap-pair blocks for the parity-split h1 layout: column block j is
    # a (128, 128) lhsT whose rows 0-63 multiply h1's even half and rows
    # 64-127 the odd half. Blocks 0-2 serve even output positions
    # ([k0;k1] [k2;k3] [k4;0]), blocks 3-5 odd ones ([0;k0] [k1;k2] [k3;k4]).
    k2t = kernel_2.transpose(1, 2, 0).astype(np.float32)  # (64, 5, 128)
    z = np.zeros((64, 128), np.float32)
    blocks = [
        np.concatenate([k2t[:, 0], k2t[:, 1]], axis=0),
        np.concatenate([k2t[:, 2], k2t[:, 3]], axis=0),
        np.concatenate([k2t[:, 4], z], axis=0),
        np.concatenate([z, k2t[:, 0]], axis=0),
        np.concatenate([k2t[:, 1], k2t[:, 2]], axis=0),
        np.concatenate([k2t[:, 3], k2t[:, 4]], axis=0),
    ]
    w2 = np.ascontiguousarray(np.concatenate(blocks, axis=1)).astype(BF16)
    w3 = np.ascontiguousarray(
        kernel_3.transpose(1, 2, 0).reshape(128, 7 * 256)
    ).astype(BF16)
    # W1 row f = c*4 + wp  ->  m1 row = wp*256 + c
    m1 = np.ascontiguousarray(
        mlp_weight_1.reshape(256, 4, 256).transpose(1, 0, 2).reshape(1024, 256)
    ).astype(BF16)
    m2 = mlp_weight_2.astype(BF16)
    m3 = mlp_weight_3.astype(BF16)

    xb = x.astype(BF16)
    in_maps = []
    for c in range(N_CORES):
        xc = xb[c * BC : (c + 1) * BC].transpose(1, 2, 0)  # (512, 20, BC)
        m = {"w1": w1, "w2": w2, "w3": w3, "m1": m1, "m2": m2, "m3": m3}
        boff = 0
        for t, bt in enumerate(TILES):
            m[f"x{t}"] = np.ascontiguousarray(
                xc[:, :, boff : boff + bt].reshape(4, 128, 20 * bt)
            )
            boff += bt
        in_maps.append(m)
    return in_maps


def run(inputs, trace=False, **kw):
    """Compile (cached), run on 8 cores, return (y_full, BassKernelResults)."""
    from concourse import bass_utils

    if "nc" not in _compiled:
        _compiled["nc"] = _build()
    nc = _compiled["nc"]
    in_maps = _prep_inputs(**inputs)
    res = bass_utils.run_bass_kernel_spmd(
        nc, in_maps, core_ids=list(range(N_CORES)), trace=trace, **kw
    )
    y = np.concatenate(
        [res.results[c]["y"].reshape(BC, 1) for c in range(N_CORES)], axis=0
    )
    return y.astype(np.float32), res


def kernel(**inputs):
    inputs = {k: np.asarray(v) for k, v in inputs.items()}
    y, _ = run(inputs)
    return y


if __name__ == "__main__":
    rng = np.random.default_rng(0)
    inputs = {
        "x": rng.standard_normal((B, E, W), dtype=np.float32),
        "kernel_1": rng.standard_normal((64, 512, 3), dtype=np.float32),
        "kernel_2": rng.standard_normal((128, 64, 5), dtype=np.float32),
        "kernel_3": rng.standard_normal((256, 128, 7), dtype=np.float32),
        "mlp_weight_1": rng.standard_normal((1024, 256), dtype=np.float32),
        "mlp_weight_2": rng.standard_normal((256, 128), dtype=np.float32),
        "mlp_weight_3": rng.standard_normal((128, 1), dtype=np.float32),
    }
    y = kernel(**inputs)
    print("out", y.shape, y.dtype, y[:4, 0])


# revision 13
# speedup vs baseline: 1.2649x; 1.0711x over previous
"""Trainium2 Bass kernel for nn_CNNFromScratch (dense 1-D CNN + MLP head).

Strategy
--------
Pure data parallelism: the batch axis (8192) is split across 8 NeuronCores
(1024 samples each); conv kernels and MLP weights are replicated.

Per core, everything is expressed as TensorE matmuls with the contraction
(input channels x taps) on the partition axis:

  - x is pre-laid-out on host as per-(tile, chunk) contiguous bf16 blocks
    (128 channels, w-major * batch free), so each chunk DMA is 128
    descriptors of 20*bt*2-byte contiguous runs (full HBM bandwidth, fast
    first-chunk arrival, ~100 descriptors/queue total).
  - All x DMAs are issued unchained in consumption order: the 16 DMA rings
    process descriptors FIFO per queue, so completions come back in issue
    order with no inter-transfer dead time (an explicit dep chain costs
    ~2.2us per link in semaphore propagation + DGE re-trigger). The bulk
    weights are gated on x-t0-q0 completion (one hop each, no W->W chain)
    so they enqueue behind x-t0 and land just before conv2-t0 needs them.
  - conv_k == sum over taps of  W_tap^T @ x[:, :, w+tap]  accumulated in PSUM.
  - Activations stay on-chip (SBUF, bf16) between layers; layout is
    (C_out partitions, w-major * batch free), which feeds the next conv's
    matmuls with plain contiguous slices.
  - maxpool = DVE tensor_max of two strided slices; MLP = accumulated
    matmuls over (channel, pooled-position) chunks.

All matmuls stream N=512 (bt=512): shorter streams cannot hide the
per-matmul LDWEIGHTS (~95ns), and exposed weight loads both waste PE time
and correlate with the PE settling at a reduced clock. Tile 0's conv1 runs
chunk-outer within position blocks so the PE starts as soon as the first
x chunk lands.

Matmul inputs are bf16 (1 cycle/row on PE), accumulation is fp32 in PSUM.
"""

import sys

sys.path.insert(0, "/opt/trn_rl_repo")

import numpy as np
import ml_dtypes

N_CORES = 8
B, E, W = 8192, 512, 20
BC = B // N_CORES  # samples per core
TILES = [512, 512]
assert sum(TILES) == BC
BT_MAX = max(TILES)

BF16 = ml_dtypes.bfloat16

_compiled = {}


def _build():
    import concourse.bass as bass
    from concourse import bacc, mybir
    import concourse.tile as tile

    dt = mybir.dt
    AF = mybir.ActivationFunctionType

    nc = bacc.Bacc(
        "TRN2",
        target_bir_lowering=False,
        debug=False,
        enable_asserts=False,
        num_devices=N_CORES,
    )

    # x: one contiguous (128, 20*bt) block per (tile, chunk), so the DMA is
    # 128 descriptors of 20*bt*2 contiguous bytes each.
    x_d = [
        nc.dram_tensor(f"x{t}", (4, 128, 20 * bt), dt.bfloat16, kind="ExternalInput").ap()
        for t, bt in enumerate(TILES)
    ]
    # Weights are packed into three 128-partition tensors (one DMA each):
    # per-instruction DMA overhead (~0.9us semaphore + DGE per transfer)
    # makes 13 separate small loads cost ~24us of queue occupancy.
    w1c_d = nc.dram_tensor("w1c", (128, 4 * 192), dt.bfloat16, kind="ExternalInput").ap()
    wa_d = nc.dram_tensor("wa", (128, 2560), dt.bfloat16, kind="ExternalInput").ap()
    wb_d = nc.dram_tensor("wb", (128, 2305), dt.bfloat16, kind="ExternalInput").ap()
    y_d = nc.dram_tensor("y", (1, BC), dt.float32, kind="ExternalOutput").ap()

    with tile.TileContext(nc) as tc:
        with (
            tc.tile_pool(name="sb", bufs=1) as sb,
            tc.tile_pool(name="ps", bufs=8, space="PSUM") as ps,
        ):
            # All input DMAs are triggered from the scalar (ACT) queue, which
            # comes out of the framework preamble several us earlier than the
            # sync queue. Issue order = ring FIFO order = consumption order:
            #   w1c, x-t0 (8 half-W slabs), wa, wb, x-t1, x-t2.
            w1c_sb = sb.tile([128, 4 * 192], dt.bfloat16, tag="w1c")
            nc.scalar.dma_start(w1c_sb[:], w1c_d[:, :])
            w1_sb = [w1c_sb[:, q * 192 : (q + 1) * 192] for q in range(4)]

            def load_x_chunk(t, q, bt, halves=False):
                xt = sb.tile(
                    [128, 20 * BT_MAX], dt.bfloat16, tag="x", bufs=5, name=f"x_{t}_{q}"
                )
                if not halves:
                    nc.scalar.dma_start(xt[:, : 20 * bt], x_d[t][q])
                return xt[:, : 20 * bt]

            # Tile 0: each chunk arrives as two half-W slabs (w0-9, w10-19)
            # into one tile; sub-tile deps let conv1's u0-3 block start as
            # soon as a chunk's first half lands (~3.6us granularity).
            x0 = [load_x_chunk(0, q, TILES[0], halves=True) for q in range(4)]
            for q in range(4):
                nc.scalar.dma_start(
                    x0[q][:, : 10 * TILES[0]], x_d[0][q][:, : 10 * TILES[0]]
                )
            for q in range(4):
                nc.scalar.dma_start(
                    x0[q][:, 10 * TILES[0] :], x_d[0][q][:, 10 * TILES[0] :]
                )

            wa_sb = sb.tile([128, 2560], dt.bfloat16, tag="wa")
            nc.scalar.dma_start(wa_sb[:], wa_d[:, :])
            w2_sb = wa_sb[:, 0:768]
            w3_sb = wa_sb[:, 768:2560]
            wb_sb = sb.tile([128, 2305], dt.bfloat16, tag="wb")
            nc.scalar.dma_start(wb_sb[:], wb_d[:, :])
            # Remaining x tiles, triggered now so their descriptors queue
            # right behind the weight packs (slot reuse deps throttle them).
            x_rest = {
                ti: [load_x_chunk(ti, q, bt) for q in range(4)]
                for ti, bt in list(enumerate(TILES))[1:]
            }
            m1_sb = [
                [wb_sb[:, (wp * 2 + q) * 256 : (wp * 2 + q + 1) * 256] for q in range(2)]
                for wp in range(4)
            ]
            m2_sb = [wb_sb[:, 2048 + q * 128 : 2048 + (q + 1) * 128] for q in range(2)]
            m3_sb = wb_sb[:, 2304:2305]

            # Warm the PE clock gate during the initial x DMA wait (dummy
            # matmuls; results never read) and pull the ACT Relu table load
            # off the critical path (emitted after the DMA triggers so it
            # doesn't block them on the ACT queue).
            warm_in = sb.tile([128, 192], dt.bfloat16, tag="warm_in")
            nc.gpsimd.memset(warm_in[:], 0.0)
            warm_ps = ps.tile([128, 512], dt.float32, tag="ps", name="warm_ps")
            for _ in range(55):
                nc.tensor.matmul(
                    warm_ps[0:64, 0:192],
                    warm_in[:, 0:64],
                    warm_in[:, :],
                    start=True,
                    stop=True,
                )
            warm_act = sb.tile([1, 1], dt.float32, tag="warm_act")
            nc.scalar.activation(warm_act[:], warm_in[0:1, 0:1], AF.Relu)

            # ---- per-batch-tile pipeline ----
            boff = 0
            for ti, bt in enumerate(TILES):
                x_sb = x0 if ti == 0 else x_rest[ti]

                # conv1: (B,512,20) -> relu -> (B,64,18)
                # Output positions are packed in pairs: even w on PSUM/SBUF
                # partitions 0-63, odd w on 64-127. The two M=64 accumulation
                # groups land on different PE column groups and execute
                # concurrently (~2x conv1 throughput).
                # The group checker doesn't model a two-col-group interleave
                # in one bank, hence skip_group_check.
                h1 = sb.tile([128, 9 * bt], dt.bfloat16, tag="h1")

                def conv1_mms(p1, u, q):
                    for k in range(3):
                        for par in range(2):
                            nc.tensor.matmul(
                                p1[par * 64 : (par + 1) * 64, :],
                                w1_sb[q][:, k * 64 : (k + 1) * 64],
                                x_sb[q][
                                    :,
                                    (2 * u + par + k) * bt : (2 * u + par + k + 1) * bt,
                                ],
                                start=(q == 0 and k == 0),
                                stop=(q == 3 and k == 2),
                                skip_group_check=True,
                            )

                if ti == 0:
                    # Chunk-outer: all matmuls for chunk q across a block of
                    # output pairs before moving to chunk q+1, so the PE
                    # starts when the first c-chunk DMA lands instead of
                    # waiting for all four. Blocks match the half-W slab
                    # split: u0-3 read only w0-9 (a chunk's first slab).
                    for u0, u1 in ((0, 4), (4, 9)):
                        p1s = [
                            ps.tile([128, bt], dt.float32, tag="ps", name=f"p1_{u}")
                            for u in range(u0, u1)
                        ]
                        for q in range(4):
                            for u in range(u0, u1):
                                conv1_mms(p1s[u - u0], u, q)
                        for u in range(u0, u1):
                            nc.vector.tensor_relu(
                                h1[:, u * bt : (u + 1) * bt], p1s[u - u0][:]
                            )
                else:
                    for u in range(9):
                        p1 = ps.tile([128, bt], dt.float32, tag="ps")
                        for q in range(4):
                            conv1_mms(p1, u, q)
                        nc.vector.tensor_relu(h1[:, u * bt : (u + 1) * bt], p1[:])

                # conv2: -> relu -> (B,128,14)
                # h1's parity-split layout lets adjacent taps fuse into one
                # full 128-row contraction (tap k on rows 0-63, tap k+1 on
                # 64-127), with zero-padded weight blocks at the edges so
                # every matmul is full-height: 3 matmuls per position
                # instead of 5. Host-prepped blocks (see _prep_inputs):
                #   even w': [k0;k1] [k2;k3] [k4;0 ]  at h1 cols t', t'+1, t'+2
                #   odd  w': [0;k0] [k1;k2] [k3;k4]   at h1 cols t', t'+1, t'+2
                h2 = sb.tile([128, 14 * bt], dt.bfloat16, tag="h2")
                for w in range(14):
                    t0 = w // 2
                    blk0 = 0 if w % 2 == 0 else 3
                    p2 = ps.tile([128, bt], dt.float32, tag="ps")
                    for j in range(3):
                        blk = blk0 + j
                        nc.tensor.matmul(
                            p2[:],
                            w2_sb[:, blk * 128 : (blk + 1) * 128],
                            h1[:, (t0 + j) * bt : (t0 + j + 1) * bt],
                            start=(j == 0),
                            stop=(j == 2),
                        )
                    nc.vector.tensor_relu(h2[:, w * bt : (w + 1) * bt], p2[:])

                # conv3: -> relu -> (B,256,8) as two 128-channel tiles
                h3 = [
                    sb.tile([128, 8 * bt], dt.bfloat16, tag=f"h3_{m}", name=f"h3_{m}")
                    for m in range(2)
                ]
                for w in range(8):
                    for m in range(2):
                        p3 = ps.tile([128, bt], dt.float32, tag="ps")
                        for k in range(7):
                            nc.tensor.matmul(
                                p3[:],
                                w3_sb[:, k * 256 + m * 128 : k * 256 + (m + 1) * 128],
                                h2[:, (w + k) * bt : (w + k + 1) * bt],
                                start=(k == 0),
                                stop=(k == 6),
                            )
                        nc.vector.tensor_relu(h3[m][:, w * bt : (w + 1) * bt], p3[:])

                # maxpool k=2 s=2: (B,256,8) -> (B,256,4)
                pooled = [
                    sb.tile([128, 4 * bt], dt.bfloat16, tag=f"pool_{m}", name=f"pool_{m}")
                    for m in range(2)
                ]
                for m in range(2):
                    for p in range(4):
                        nc.vector.tensor_max(
                            pooled[m][:, p * bt : (p + 1) * bt],
                            h3[m][:, (2 * p) * bt : (2 * p + 1) * bt],
                            h3[m][:, (2 * p + 1) * bt : (2 * p + 2) * bt],
                        )

                # mlp1: (B,1024)->(B,256), f = c*4 + wp
                g1 = [
                    sb.tile([128, bt], dt.bfloat16, tag=f"g1_{j}", name=f"g1_{j}")
                    for j in range(2)
                ]
                for j in range(2):
                    pm = ps.tile([128, bt], dt.float32, tag="ps")
                    for wp in range(4):
                        for q in range(2):
                            nc.tensor.matmul(
                                pm[:],
                                m1_sb[wp][q][:, j * 128 : (j + 1) * 128],
                                pooled[q][:, wp * bt : (wp + 1) * bt],
                                start=(wp == 0 and q == 0),
                                stop=(wp == 3 and q == 1),
                            )
                    nc.vector.tensor_relu(g1[j][:], pm[:])

                # mlp2: (B,256)->(B,128)
                g2 = sb.tile([128, bt], dt.bfloat16, tag="g2")
                pm = ps.tile([128, bt], dt.float32, tag="ps")
                for q in range(2):
                    nc.tensor.matmul(
                        pm[:], m2_sb[q][:], g1[q][:], start=(q == 0), stop=(q == 1)
                    )
                nc.vector.tensor_relu(g2[:], pm[:])

                # mlp3: (B,128)->(B,1)
                pm = ps.tile([1, bt], dt.float32, tag="ps")
                nc.tensor.matmul(pm[:], m3_sb[:], g2[:], start=True, stop=True)
                y_sb = sb.tile([1, BT_MAX], dt.float32, tag="y_sb", bufs=2)
                nc.vector.tensor_copy(y_sb[:, :bt], pm[:])
                nc.sync.dma_start(y_d[:, boff : boff + bt], y_sb[:, :bt])

                boff += bt

    nc.compile()
    return nc


def _prep_inputs(x, kernel_1, kernel_2, kernel_3, mlp_weight_1, mlp_weight_2, mlp_weight_3):
    """Host-side sharding + layout prep. Returns in_maps (one dict per core)."""
    # w1c[p, q*192 + col] = w1[q*128 + p, col]  (4 channel-chunks side by side)
    w1 = kernel_1.transpose(1, 2, 0).reshape(512, 3 * 64)
    w1c = np.ascontiguousarray(
        w1.reshape(4, 128, 192).transpose(1, 0, 2).reshape(128, 4 * 192)
    ).astype(BF16)
    # conv2 tap-pair blocks for the parity-split h1 layout: column block j is
    # a (128, 128) lhsT whose rows 0-63 multiply h1's even half and rows
    # 64-127 the odd half. Blocks 0-2 serve even output positions
    # ([k0;k1] [k2;k3] [k4;0]), blocks 3-5 odd ones ([0;k0] [k1;k2] [k3;k4]).
    k2t = kernel_2.transpose(1, 2, 0).astype(np.float32)  # (64, 5, 128)
    z = np.zeros((64, 128), np.float32)
    blocks = [
        np.concatenate([k2t[:, 0], k2t[:, 1]], axis=0),
        np.concatenate([k2t[:, 2], k2t[:, 3]], axis=0),
        np.concatenate([k2t[:, 4], z], axis=0),
        np.concatenate([z, k2t[:, 0]], axis=0),
        np.concatenate([k2t[:, 1], k2t[:, 2]], axis=0),
        np.concatenate([k2t[:, 3], k2t[:, 4]], axis=0),
    ]
    w2 = np.concatenate(blocks, axis=1)  # (128, 768)
    w3 = kernel_3.transpose(1, 2, 0).reshape(128, 7 * 256)
    wa = np.ascontiguousarray(np.concatenate([w2, w3], axis=1)).astype(BF16)
    # W1 row f = c*4 + wp -> m1 rows (wp, q, c) packed as wb columns
    # (wp*2+q)*256 + j; m2 as columns 2048 + q*128 + j; m3 at 2304.
    m1 = mlp_weight_1.reshape(256, 4, 256).transpose(1, 0, 2).reshape(1024, 256)
    m1b = m1.reshape(4, 2, 128, 256).transpose(2, 0, 1, 3).reshape(128, 2048)
    m2b = mlp_weight_2.reshape(2, 128, 128).transpose(1, 0, 2).reshape(128, 256)
    wb = np.ascontiguousarray(
        np.concatenate([m1b, m2b, mlp_weight_3], axis=1)
    ).astype(BF16)

    xb = x.astype(BF16)
    in_maps = []
    for c in range(N_CORES):
        xc = xb[c * BC : (c + 1) * BC].transpose(1, 2, 0)  # (512, 20, BC)
        m = {"w1c": w1c, "wa": wa, "wb": wb}
        boff = 0
        for t, bt in enumerate(TILES):
            m[f"x{t}"] = np.ascontiguousarray(
                xc[:, :, boff : boff + bt].reshape(4, 128, 20 * bt)
            )
            boff += bt
        in_maps.append(m)
    return in_maps


def run(inputs, trace=False, **kw):
    """Compile (cached), run on 8 cores, return (y_full, BassKernelResults)."""
    from concourse import bass_utils

    if "nc" not in _compiled:
        _compiled["nc"] = _build()
    nc = _compiled["nc"]
    in_maps = _prep_inputs(**inputs)
    res = bass_utils.run_bass_kernel_spmd(
        nc, in_maps, core_ids=list(range(N_CORES)), trace=trace, **kw
    )
    y = np.concatenate(
        [res.results[c]["y"].reshape(BC, 1) for c in range(N_CORES)], axis=0
    )
    return y.astype(np.float32), res


def kernel(**inputs):
    inputs = {k: np.asarray(v) for k, v in inputs.items()}
    y, _ = run(inputs)
    return y


if __name__ == "__main__":
    rng = np.random.default_rng(0)
    inputs = {
        "x": rng.standard_normal((B, E, W), dtype=np.float32),
        "kernel_1": rng.standard_normal((64, 512, 3), dtype=np.float32),
        "kernel_2": rng.standard_normal((128, 64, 5), dtype=np.float32),
        "kernel_3": rng.standard_normal((256, 128, 7), dtype=np.float32),
        "mlp_weight_1": rng.standard_normal((1024, 256), dtype=np.float32),
        "mlp_weight_2": rng.standard_normal((256, 128), dtype=np.float32),
        "mlp_weight_3": rng.standard_normal((128, 1), dtype=np.float32),
    }
    y = kernel(**inputs)
    print("out", y.shape, y.dtype, y[:4, 0])
